# revision 1
# baseline (speedup 1.0000x reference)
"""Trainium2 Bass kernel for CharacteristicFunctionNetwork.

Computes, for full inputs (see shapes below):
    feats[o,p,i] = mean_j cos(wm[o,p] * adj[o,i,j])        # o<3, p<16, i,j<2048
    ms = feats transposed/reshaped to [n, 48]
    h1 = relu(ms @ w1 + b1); h2 = relu(h1 @ w2 + b2)
    abstract = tanh(h2 @ p1 + pb1); att = softmax(abstract @ p2 + pb2, axis=0)
    g = (att.T @ h2).reshape(1, -1); out = log_softmax(g @ cw + cb)

Strategy (8 NeuronCores, SPMD):
  - Shard adj rows (nodes) across cores: 256 rows/core for each of 3 orders.
  - cos in "turns": t = a*(|w|/2pi) + 1/4, k = (t+M)-M (round-to-int trick,
    M = 1.5*2^23), d = t - k in [-0.5, 0.5], cos(a*w) = sin(2*pi*d).
    walrus has no mod ALU op, so range reduction is round+subtract:
    t on ScalarE (Identity, scale AP) for most points / DVE for the rest,
    k on DVE (dual add/sub tensor_scalar, 2x mode), d on DVE tensor_tensor,
    sin on ScalarE with the row-sum fused via accum_out.
  - Per-core rows run the tiny MLP locally in transposed layout; the pooling
    softmax needs only a global sum of exp-weighted partials: AllReduce of
    a [8, 33] tile (P = e^T @ h2 partials and z = sum e). exp is computed
    without max-subtraction (|s| <= ~3, safe in fp32).
  - Every core finishes the classifier redundantly; core 0's output is used.
"""

import os

import numpy as np

ORDER, PTS, N = 3, 16, 2048
NCORES = 8
RPC = N // NCORES  # rows per core (256)
NCHUNK = RPC // 128  # 128-row chunks per core (2)
D1, D2, POOL1, POOL2, LABELS = 64, 32, 32, 8, 10
K = ORDER * PTS  # 48

_STATE = {}

# engine-assignment knobs (A/B-tested on HW; defaults = best measured)
# ts1 is split within each pair along the free axis: ScalarE computes
# columns [:ACT_COLS], DVE computes [ACT_COLS:], balancing both engines.
ACT_COLS = int(os.environ.get("KERNEL_ACT_COLS", "1600"))
GPS_ROUND = float(os.environ.get("KERNEL_GPS_ROUND", "0.0"))  # frac of rounds on GpSimd
TKD_BUFS = int(os.environ.get("KERNEL_TKD_BUFS", "3"))


def _build():
    import concourse.bacc as bacc
    import concourse.mybir as mybir
    import concourse.tile as tile

    F32 = mybir.dt.float32
    AF = mybir.ActivationFunctionType
    ALU = mybir.AluOpType
    TWO_PI = float(2 * np.pi)
    RND = float(1.5 * 2**23)  # add/sub forces round-to-nearest-int in fp32

    nc = bacc.Bacc("TRN2", target_bir_lowering=False, debug=False,
                   num_devices=NCORES)

    adj_s = nc.dram_tensor("adj_s", [ORDER, RPC, N], F32, kind="ExternalInput").ap()
    wturns = nc.dram_tensor("wturns", [1, K], F32, kind="ExternalInput").ap()
    w1 = nc.dram_tensor("w1", [K, D1], F32, kind="ExternalInput").ap()
    b1 = nc.dram_tensor("b1", [D1, 1], F32, kind="ExternalInput").ap()
    w2 = nc.dram_tensor("w2", [D1, D2], F32, kind="ExternalInput").ap()
    b2 = nc.dram_tensor("b2", [D2, 1], F32, kind="ExternalInput").ap()
    p1 = nc.dram_tensor("p1", [D2, POOL1], F32, kind="ExternalInput").ap()
    pb1 = nc.dram_tensor("pb1", [POOL1, 1], F32, kind="ExternalInput").ap()
    p2 = nc.dram_tensor("p2", [POOL1, POOL2], F32, kind="ExternalInput").ap()
    pb2 = nc.dram_tensor("pb2", [POOL2, 1], F32, kind="ExternalInput").ap()
    cwS = nc.dram_tensor("cwS", [D2, POOL2 * LABELS], F32, kind="ExternalInput").ap()
    cb = nc.dram_tensor("cb", [1, LABELS], F32, kind="ExternalInput").ap()
    ident = nc.dram_tensor("ident", [128, 128], F32, kind="ExternalInput").ap()
    out = nc.dram_tensor("out", [1, LABELS], F32, kind="ExternalOutput").ap()

    with tile.TileContext(nc) as tc:
        with (
            tc.tile_pool(name="const", bufs=1) as const,
            tc.tile_pool(name="work", bufs=2) as work,
            tc.tile_pool(name="small", bufs=1) as small,
            tc.tile_pool(name="ep", bufs=1, space="PSUM") as ep,
            tc.tile_pool(name="ep2", bufs=1, space="PSUM") as ep2,
            tc.tile_pool(name="dram", bufs=1, space="DRAM") as dram,
        ):
            # ---- prefetch the first orders' adjacency on the HWDGE queue
            a_tiles = {}

            def load_a(o):
                a = work.tile([128, NCHUNK, N], F32, name=f"a{o}", tag="a")
                for c in range(NCHUNK):
                    nc.sync.dma_start(a[:, c, :],
                                      adj_s[o, c * 128:(c + 1) * 128, :])
                a_tiles[o] = a

            load_a(0)
            if ORDER > 1:
                load_a(1)

            # ---- constants (SWDGE queue, parallel to the big loads) ----
            idt = const.tile([128, 128], F32)
            nc.sync.dma_start(idt[:], ident[:])
            wrow = const.tile([1, K], F32)
            nc.sync.dma_start(wrow[:], wturns[:])
            ones = const.tile([1, 128], F32)
            nc.vector.memset(ones[:], 1.0)
            quarter = const.tile([128, 1], F32)
            nc.vector.memset(quarter[:], 0.25)
            zerot = const.tile([128, 1], F32)
            nc.vector.memset(zerot[:], 0.0)
            w1t = const.tile([K, D1], F32)
            nc.sync.dma_start(w1t[:], w1[:])
            b1t = const.tile([D1, 1], F32)
            nc.sync.dma_start(b1t[:], b1[:])
            w2t = const.tile([D1, D2], F32)
            nc.sync.dma_start(w2t[:], w2[:])
            b2t = const.tile([D2, 1], F32)
            nc.sync.dma_start(b2t[:], b2[:])
            p1t = const.tile([D2, POOL1], F32)
            nc.sync.dma_start(p1t[:], p1[:])
            pb1t = const.tile([POOL1, 1], F32)
            nc.sync.dma_start(pb1t[:], pb1[:])
            p2t = const.tile([POOL1, POOL2], F32)
            nc.sync.dma_start(p2t[:], p2[:])
            pb2t = const.tile([POOL2, 1], F32)
            nc.sync.dma_start(pb2t[:], pb2[:])
            cwt = const.tile([D2, POOL2 * LABELS], F32)
            nc.sync.dma_start(cwt[:], cwS[:])
            cbt = const.tile([1, LABELS], F32)
            nc.sync.dma_start(cbt[:], cb[:])

            # broadcast |w|/2pi across 128 partitions: [128, 48] via ones ⊗ wrow
            with tc.tile_pool(name="bootp", bufs=1, space="PSUM") as bootp:
                wbp = bootp.tile([128, K], F32)
                nc.tensor.matmul(wbp[:], ones[:], wrow[:], start=True, stop=True)
                wt = const.tile([128, K], F32)
                nc.vector.tensor_copy(wt[:], wbp[:])

            # warm up the collective path (CC library load + ring setup)
            # during the main loop so the real AllReduce at the tail is fast
            warm_sb = small.tile([POOL2, D2 + 1], F32)
            nc.vector.memset(warm_sb[:], 0.0)
            ccw_in = dram.tile([POOL2, D2 + 1], F32)
            ccw_out = dram.tile([POOL2, D2 + 1], F32)
            nc.sync.dma_start(ccw_in[:], warm_sb[:])
            nc.gpsimd.collective_compute(
                "AllReduce",
                mybir.AluOpType.add,
                replica_groups=[list(range(NCORES))],
                ins=[ccw_in.opt()],
                outs=[ccw_out.opt()],
            )

            # ---- main loop: feats for this core's rows ----
            # row-sum[k, i_local] of cos(w_k * adj[o, i, j]), k = o*16+p.
            # partition p holds rows {p, 128+p}; free axis = (chunk, j).
            ms_chunks = [small.tile([128, K], F32, name=f"ms{c}", tag=f"ms{c}")
                         for c in range(NCHUNK)]
            pairs = [(o, p) for o in range(ORDER) for p in range(PTS)]

            def emit_t(idx):
                # t = a*(|w|/2pi) + 1/4, free-axis-split across ScalarE/DVE
                o, p = pairs[idx]
                col = o * PTS + p
                t = work.tile([128, NCHUNK, N], F32, name=f"t{idx}", tag="t",
                              bufs=TKD_BUFS)
                X = ACT_COLS
                if X > 0:
                    nc.scalar.activation(t[:, :, :X], a_tiles[o][:, :, :X],
                                         AF.Identity, bias=quarter[:],
                                         scale=wt[:, col:col + 1])
                if X < N:
                    nc.vector.tensor_scalar(
                        t[:, :, X:], a_tiles[o][:, :, X:],
                        wt[:, col:col + 1], 0.25, ALU.mult, ALU.add)
                return t

            t_cur = emit_t(0)
            for idx, (o, p) in enumerate(pairs):
                col = o * PTS + p
                if p == 0 and o >= 1 and o + 1 < ORDER:
                    load_a(o + 1)  # prefetch next order's rows
                # emit next pair's t first so ScalarE stays ahead of DVE
                t_next = emit_t(idx + 1) if idx + 1 < len(pairs) else None
                # k = round(t); d = t - k in [-0.5, 0.5]
                k = work.tile([128, NCHUNK, N], F32, name=f"k{idx}", tag="k",
                              bufs=2)
                d = work.tile([128, NCHUNK, N], F32, name=f"d{idx}", tag="d",
                              bufs=TKD_BUFS)
                if (idx * 7919) % 48 / 48.0 < GPS_ROUND:
                    nc.gpsimd.tensor_scalar(k[:], t_cur[:], RND, RND,
                                            ALU.add, ALU.subtract)
                else:
                    nc.vector.tensor_scalar(k[:], t_cur[:], RND, RND,
                                            ALU.add, ALU.subtract)
                nc.vector.tensor_tensor(d[:], t_cur[:], k[:], ALU.subtract)
                # sin(2pi*d) = cos(a*w); accumulate over j per chunk
                for c in range(NCHUNK):
                    s = work.tile([128, N], F32, name=f"s{idx}_{c}", tag="s", bufs=1)
                    nc.scalar.activation(
                        s[:], d[:, c, :], AF.Sin, bias=zerot[:],
                        scale=TWO_PI,
                        accum_out=ms_chunks[c][:, col:col + 1])
                t_cur = t_next

            # scale row-sums to means and transpose into mst [K, RPC]
            mst = small.tile([K, RPC], F32)
            for c in range(NCHUNK):
                msn = work.tile([128, K], F32, tag="msn")
                nc.vector.tensor_scalar(
                    msn[:], ms_chunks[c][:], 1.0 / N, None, ALU.mult)
                t1 = ep.tile([K, 128], F32, tag="t1")
                nc.tensor.transpose(t1[:], msn[:], idt[:])
                nc.vector.tensor_copy(mst[:, c * 128:(c + 1) * 128], t1[:])

            # ---- local MLP in transposed layout ([feat, row]) ----
            h1p = ep.tile([D1, RPC], F32, tag="ps2")
            nc.tensor.matmul(h1p[:], w1t[:], mst[:], start=True, stop=True)
            h1 = small.tile([D1, RPC], F32)
            nc.scalar.activation(h1[:], h1p[:], AF.Relu, bias=b1t[:], scale=1.0)

            h2p = ep.tile([D2, RPC], F32, tag="ps2")
            nc.tensor.matmul(h2p[:], w2t[:], h1[:], start=True, stop=True)
            h2 = small.tile([D2, RPC], F32)
            nc.scalar.activation(h2[:], h2p[:], AF.Relu, bias=b2t[:], scale=1.0)

            abp = ep.tile([POOL1, RPC], F32, tag="ps2")
            nc.tensor.matmul(abp[:], p1t[:], h2[:], start=True, stop=True)
            ab = small.tile([POOL1, RPC], F32)
            nc.scalar.activation(ab[:], abp[:], AF.Tanh, bias=pb1t[:], scale=1.0)

            sp = ep.tile([POOL2, RPC], F32, tag="ps2")
            nc.tensor.matmul(sp[:], p2t[:], ab[:], start=True, stop=True)
            # e = exp(s + pb2), z = row-sums of e (softmax without max-shift;
            # |s| <= ~3 so fp32 exp is safe)
            e = small.tile([POOL2, RPC], F32)
            z8 = small.tile([POOL2, 1], F32)
            nc.scalar.activation(e[:], sp[:], AF.Exp, bias=pb2t[:], scale=1.0,
                                 accum_out=z8[:])
            # dummy Ln: pulls the natural_log_exp table load into the
            # AllReduce wait window instead of the serial post-collective tail
            lnw = small.tile([1, 1], F32)
            nc.scalar.activation(lnw[:], z8[:1, :], AF.Ln, bias=0.0, scale=1.0)

            # P = e^T stacked against h2: pp[j, d] = sum_i e[j,i] * h2[d,i]
            pp = ep2.tile([POOL2, D2], F32, tag="pp")
            for c in range(NCHUNK):
                etp = ep.tile([128, POOL2], F32, tag="et")
                nc.tensor.transpose(etp[:], e[:, c * 128:(c + 1) * 128],
                                    idt[:POOL2, :POOL2])
                ets = work.tile([128, POOL2], F32, tag="ets")
                nc.vector.tensor_copy(ets[:], etp[:])
                htp = ep.tile([128, D2], F32, tag="ht")
                nc.tensor.transpose(htp[:], h2[:, c * 128:(c + 1) * 128],
                                    idt[:D2, :D2])
                hts = work.tile([128, D2], F32, tag="hts")
                nc.vector.tensor_copy(hts[:], htp[:])
                nc.tensor.matmul(pp[:], ets[:], hts[:],
                                 start=(c == 0), stop=(c == NCHUNK - 1))

            # pack [P | z] into [8, 33] and AllReduce across cores
            comb = small.tile([POOL2, D2 + 1], F32)
            nc.vector.tensor_copy(comb[:, :D2], pp[:])
            nc.vector.tensor_copy(comb[:, D2:D2 + 1], z8[:])
            ccin = dram.tile([POOL2, D2 + 1], F32)
            ccout = dram.tile([POOL2, D2 + 1], F32)
            nc.sync.dma_start(ccin[:], comb[:])
            nc.gpsimd.collective_compute(
                "AllReduce",
                mybir.AluOpType.add,
                replica_groups=[list(range(NCORES))],
                ins=[ccin.opt()],
                outs=[ccout.opt()],
            )
            r = small.tile([POOL2, D2 + 1], F32)
            nc.sync.dma_start(r[:], ccout[:])

            # g[j, d] = P[j, d] / z[j]
            rz = small.tile([POOL2, 1], F32)
            nc.vector.reciprocal(rz[:], r[:, D2:D2 + 1])
            g = small.tile([POOL2, D2], F32)
            nc.scalar.activation(g[:], r[:, :D2], AF.Copy, bias=0.0, scale=rz[:])

            # logits[l] = sum_j sum_d g[j,d] cw[j*32+d, l] + cb[l]
            gtp = ep.tile([D2, POOL2], F32, tag="gt")
            nc.tensor.transpose(gtp[:], g[:], idt[:POOL2, :POOL2])
            gt = small.tile([D2, POOL2], F32)
            nc.vector.tensor_copy(gt[:], gtp[:])
            logp = ep2.tile([1, LABELS], F32, tag="logp")
            for j in range(POOL2):
                nc.tensor.matmul(logp[:], gt[:, j:j + 1],
                                 cwt[:, j * LABELS:(j + 1) * LABELS],
                                 start=(j == 0), stop=(j == POOL2 - 1))
            lg = small.tile([1, LABELS], F32)
            nc.vector.tensor_tensor(lg[:], logp[:], cbt[:], ALU.add)

            # log_softmax over the 10 logits (|logits| ~ 2, no max-shift
            # needed in fp32; matches reference to ~1e-7)
            u10 = lg
            e10 = small.tile([1, LABELS], F32)
            z1 = small.tile([1, 1], F32)
            nc.scalar.activation(e10[:], u10[:], AF.Exp, bias=zerot[:1, :],
                                 scale=1.0, accum_out=z1[:])
            lnz = small.tile([1, 1], F32)
            nc.scalar.activation(lnz[:], z1[:], AF.Ln, bias=0.0, scale=1.0)
            nlnz = small.tile([1, 1], F32)
            nc.vector.tensor_scalar(nlnz[:], lnz[:], -1.0, None, ALU.mult)
            o10 = small.tile([1, LABELS], F32)
            nc.scalar.activation(o10[:], u10[:], AF.Identity, bias=nlnz[:],
                                 scale=1.0)
            nc.sync.dma_start(out[:], o10[:])

    nc.compile()
    return nc


def get_module():
    if "nc" not in _STATE:
        _STATE["nc"] = _build()
    return _STATE["nc"]


def make_in_maps(inputs):
    adj = np.asarray(inputs["adj"], np.float32)
    wm = np.asarray(inputs["wm"], np.float32)
    base = {
        "wturns": np.ascontiguousarray(
            (np.abs(wm).astype(np.float64) / (2 * np.pi))
            .astype(np.float32).reshape(1, K)),
        "w1": np.ascontiguousarray(np.asarray(inputs["w1"], np.float32)),
        "b1": np.ascontiguousarray(np.asarray(inputs["b1"], np.float32).reshape(D1, 1)),
        "w2": np.ascontiguousarray(np.asarray(inputs["w2"], np.float32)),
        "b2": np.ascontiguousarray(np.asarray(inputs["b2"], np.float32).reshape(D2, 1)),
        "p1": np.ascontiguousarray(np.asarray(inputs["p1"], np.float32)),
        "pb1": np.ascontiguousarray(np.asarray(inputs["pb1"], np.float32).reshape(POOL1, 1)),
        "p2": np.ascontiguousarray(np.asarray(inputs["p2"], np.float32)),
        "pb2": np.ascontiguousarray(np.asarray(inputs["pb2"], np.float32).reshape(POOL2, 1)),
        "cwS": np.ascontiguousarray(
            np.asarray(inputs["cw"], np.float32)
            .reshape(POOL2, D2, LABELS).transpose(1, 0, 2).reshape(D2, POOL2 * LABELS)),
        "cb": np.ascontiguousarray(np.asarray(inputs["cb"], np.float32).reshape(1, LABELS)),
        "ident": np.eye(128, dtype=np.float32),
    }
    in_maps = []
    for c in range(NCORES):
        m = dict(base)
        m["adj_s"] = np.ascontiguousarray(adj[:, c * RPC:(c + 1) * RPC, :])
        in_maps.append(m)
    return in_maps


def kernel(**inputs) -> np.ndarray:
    nc = get_module()
    in_maps = make_in_maps(inputs)
    from concourse.bass_utils import run_bass_kernel_spmd

    res = run_bass_kernel_spmd(nc, in_maps, list(range(NCORES)))
    return np.asarray(res.results[0]["out"], np.float32).reshape(1, LABELS)

